# revision 1
# baseline (speedup 1.0000x reference)
"""Trainium2 Bass kernel for nn_Classifier_22299470201420 (retrieval_knn).

Reference computation:
    hv   = (samples - 0.5) @ W.T          # [B, D] random projection
    bip  = where(hv > 0, 1, -1)           # bipolar hypervector
    dots = bip @ (2*centroids - 1).T      # [B, C] bipolar dot products
    sim  = int32(0.5 * (D + dots))        # hamming similarity counts

Sharding: data-parallel over the batch dim — each of the 8 cores gets
B/8 = 512 samples; W and centroids are replicated (no collectives).

Device kernel (per core), all matmuls in bf16 on the PE:
  - samples-0.5 is split on the host into bf16 hi + lo parts (hi+lo
    reproduces fp32 precision; W is exactly representable in bf16), so
    hv accumulates in fp32 PSUM with fp32-level accuracy at bf16 speed.
  - matmul1 produces hv^T tiles [d=128, b=512]; ScalarE Sign() turns them
    into bipolar bf16 tiles; matmul2 accumulates all 79 d-tiles into one
    PSUM bank [100, 512] of partial dot products.
  - D is zero-padded 10000 -> 10112 (79*128) in both W^T and centroids^T,
    so padded dims contribute exactly 0 to the dots.
  - The final affine 0.5*(D+dots) + int32 cast + transpose happens on the
    host on the tiny [100, 512] per-core outputs.
"""

import numpy as np
import ml_dtypes

B, F, D, C = 4096, 1024, 10000, 100
NCORES = 8
BC = B // NCORES          # samples per core
NT = 79                   # number of 128-wide d tiles
DPAD = NT * 128           # 10112
FG = F // 128             # 8 f-chunks of 128

bf16 = ml_dtypes.bfloat16

_prog_cache = {}


def _build_program():
    if "nc" in _prog_cache:
        return _prog_cache["nc"]

    from contextlib import ExitStack
    import concourse.bacc as bacc
    import concourse.tile as tile
    import concourse.mybir as mybir

    mbf16 = mybir.dt.bfloat16
    mf32 = mybir.dt.float32

    nc = bacc.Bacc("TRN2", target_bir_lowering=False, debug=False)

    st_hi_d = nc.dram_tensor("st_hi", [128, FG * BC], mbf16, kind="ExternalInput")
    st_lo_d = nc.dram_tensor("st_lo", [128, FG * BC], mbf16, kind="ExternalInput")
    wt_d = nc.dram_tensor("wt", [NT, 128, FG * 128], mbf16, kind="ExternalInput")
    cb_d = nc.dram_tensor("cb", [128, NT * C], mbf16, kind="ExternalInput")
    dots_d = nc.dram_tensor("dots", [C, BC], mf32, kind="ExternalOutput")

    with tile.TileContext(nc) as tc, ExitStack() as ctx:
        const = ctx.enter_context(tc.tile_pool(name="const", bufs=1))
        wtp = ctx.enter_context(tc.tile_pool(name="wtp", bufs=8))
        hvp = ctx.enter_context(tc.tile_pool(name="hvp", bufs=4, space="PSUM"))
        dotsp = ctx.enter_context(tc.tile_pool(name="dotsp", bufs=1, space="PSUM"))
        bipp = ctx.enter_context(tc.tile_pool(name="bipp", bufs=4))

        st_hi = const.tile([128, FG * BC], mbf16, tag="st_hi")
        nc.sync.dma_start(st_hi[:], st_hi_d[:])
        st_lo = const.tile([128, FG * BC], mbf16, tag="st_lo")
        nc.sync.dma_start(st_lo[:], st_lo_d[:])
        cb = const.tile([128, NT * C], mbf16, tag="cb")
        nc.sync.dma_start(cb[:], cb_d[:])

        pd = dotsp.tile([C, BC], mf32)

        for dt in range(NT):
            wt = wtp.tile([128, FG * 128], mbf16)
            nc.sync.dma_start(wt[:], wt_d[dt])
            ph = hvp.tile([128, BC], mf32)
            for g in range(FG):
                lw = wt[:, g * 128 : (g + 1) * 128]
                nc.tensor.matmul(
                    ph[:], lhsT=lw, rhs=st_hi[:, g * BC : (g + 1) * BC],
                    start=(g == 0), stop=False,
                )
                nc.tensor.matmul(
                    ph[:], lhsT=lw, rhs=st_lo[:, g * BC : (g + 1) * BC],
                    start=False, stop=(g == FG - 1),
                )
            bip = bipp.tile([128, BC], mbf16)
            nc.scalar.activation(bip[:], ph[:], mybir.ActivationFunctionType.Sign)
            nc.tensor.matmul(
                pd[:], lhsT=cb[:, dt * C : (dt + 1) * C], rhs=bip[:],
                start=(dt == 0), stop=(dt == NT - 1),
            )

        out_sb = const.tile([C, BC], mf32, tag="out_sb")
        nc.scalar.copy(out_sb[:], pd[:])
        nc.sync.dma_start(dots_d[:], out_sb[:])

    nc.compile()
    _prog_cache["nc"] = nc
    return nc


def _pack_shared(W, centroids):
    # W^T padded [F, DPAD], packed so each d-tile is one contiguous
    # [128, 1024] SBUF image: packed[dt, p, g*128+j] = W^T[g*128+p, dt*128+j]
    WT = np.zeros((F, DPAD), dtype=bf16)
    WT[:, :D] = W.astype(bf16).T
    wt_packed = np.ascontiguousarray(
        WT.reshape(FG, 128, NT, 128).transpose(2, 1, 0, 3).reshape(NT, 128, FG * 128)
    )
    # centroids^T (bipolar) padded [DPAD, C]: packed[p, t*C+c] = cb^T[t*128+p, c]
    cbT = np.zeros((DPAD, C), dtype=bf16)
    cbT[:D, :] = (2.0 * centroids.astype(np.float32) - 1.0).astype(bf16).T
    cb_packed = np.ascontiguousarray(
        cbT.reshape(NT, 128, C).transpose(1, 0, 2).reshape(128, NT * C)
    )
    return wt_packed, cb_packed


def _pack_st(part_c):
    # part_c: [BC, F] bf16 -> packed[p, g*BC+b] = part_c.T[g*128+p, b]
    return np.ascontiguousarray(
        part_c.T.reshape(FG, 128, BC).transpose(1, 0, 2).reshape(128, FG * BC)
    )


def _run(inputs, trace=False):
    from concourse.bass_utils import run_bass_kernel_spmd

    samples = np.asarray(inputs["samples"], dtype=np.float32)
    W = np.asarray(inputs["W"], dtype=np.float32)
    centroids = np.asarray(inputs["centroids"], dtype=np.float32)
    assert samples.shape == (B, F) and W.shape == (D, F) and centroids.shape == (C, D)

    x = samples - 0.5
    hi = x.astype(bf16)
    lo = (x - hi.astype(np.float32)).astype(bf16)
    wt_packed, cb_packed = _pack_shared(W, centroids)

    in_maps = []
    for i in range(NCORES):
        sl = slice(i * BC, (i + 1) * BC)
        in_maps.append(
            {
                "st_hi": _pack_st(hi[sl]),
                "st_lo": _pack_st(lo[sl]),
                "wt": wt_packed,
                "cb": cb_packed,
            }
        )

    nc = _build_program()
    res = run_bass_kernel_spmd(nc, in_maps, list(range(NCORES)), trace=trace)

    out = np.empty((B, C), dtype=np.int32)
    for i in range(NCORES):
        dots = np.asarray(res.results[i]["dots"], dtype=np.float32)  # [C, BC]
        sim = np.rint(0.5 * (np.float64(D) + dots.astype(np.float64)))
        out[i * BC : (i + 1) * BC, :] = sim.T.astype(np.int32)
    return out, res


def kernel(samples, W, centroids):
    out, _ = _run({"samples": samples, "W": W, "centroids": centroids})
    return out
